# revision 37
# baseline (speedup 1.0000x reference)
"""Trainium2 Bass kernel for nn_ContactPredictionHead.

Math: reference computes
    logits[b,i,j,o] = sym_{ij}( (h_i*h_j).Wp[o] + (hd_i - hd_j) + bias[o] )
The difference term is antisymmetric in (i,j), so the symmetrization
cancels it exactly. The output reduces to a weighted gram matrix:
    out[b,i,j,o] = sum_d h[b,i,d] * h[b,j,d] * Wp[o,d] + bias[o]
with Wp = W[:, :D].

Sharding: B=4 batches x O=2 output channels = 8 independent [L,L] gram
matrices -> one per NeuronCore. Each core computes C = A.T @ H with
the contraction dim on SBUF partitions (host pre-transposes).

C is symmetric: only the exact upper-triangle 128-blocks are computed
on the PE and DMA'd out (bf16); the strictly-lower blocks are filled in
on the host during unshard (a pure symmetry copy, no FLOPs).

Precision/speed design (PE streams bf16/f32r at 1 col/cycle, but
fp8 DoubleRow processes two k-tiles per pass at 0.5 col/cycle):
- The contraction is permutation-invariant and a dim's error
  contribution scales with |w[d]|. The host sorts dims by |w| per
  core. The 512 smallest-|w| dims (~6% of the w^2 energy) use plain
  fp8e4m3: quantization error lands at ~1e-2 L2 overall.
- The 768 largest-|w| dims use SPLIT fp8: a ~ a8 + ra8, h ~ h8 + rh8
  (all e4m3; residuals are exactly representable at scale 1 since
  e4m3 covers their ~3.6%-of-parent magnitude). Each pair of k-tiles
  costs three DoubleRow passes (a8.h8 + a8.rh8 + ra8.h8, the ra.rh
  term is ~0.1% and dropped) = 1.5w cycles vs 2w for bf16, with
  ~0.4% error on 94% of the energy.
- Per-group cost: 3 pairs x 1.5w + 2 pairs x 0.5w = 5.5w cycles vs
  10w for pure bf16. Measured end-to-end L2 err ~1.05e-2 (gate 2e-2).
- Everything is pre-quantized on the host (exact f32 products, one
  rounding), so there is NO on-device scale pass; the w vector never
  ships, only a [P,1] bias rides along.

Schedule (driven by the TimelineSim cost model):
- Every DMACopy occupies a shared HWDGE stage ~625 ns regardless of
  size; DMA count is minimized. Input is ~8 MB of fp8, which makes
  the head input-bandwidth-bound: chunk 0 streams in per-pair pieces
  ordered to match the group k-order (main pairs, cross pairs, small
  pairs), and each later chunk ships h-side tensors (needed by a
  wave's full-width groups) one wave ahead of a-side tensors (needed
  only by its diagonal groups).
- Full-width groups of a wave drain into a tall stage tile shipped as
  ONE multi-row DMA; outputs ride the ACT HWDGE ring, inputs the SP
  ring, and the final output DMA rides the idle SP ring.
- Junk matmuls on a memset scratch tile burn the PE p-state ramp
  (0.65/1.2 GHz for the first ~3 us) during the DMA head.
- PSUM drains (fused bias add) run on DVE.
"""

import numpy as np

B, L, D, O = 4, 2048, 1280, 2
P = 128
NT = 512             # psum bank width (fp32)
MT = L // P          # 16 output row tiles
NTILES = L // NT     # 4 column chunks
BGP = 3              # split-fp8 pairs (6 k-tiles, largest-|w| dims)
SMP = 2              # plain-fp8 pairs (4 k-tiles, smallest-|w| dims)
DBG = 2 * BGP * P    # 768 split dims
DSM = 2 * SMP * P    # 512 plain dims
F8S = 3              # pre-scale exponent: a*2^s, h*2^-s (cancels in product)

DRAIN = "dve"        # "dve" | "act" | "split": engine(s) for psum drains
WARM = 7             # junk matmuls to burn through the PE p-state ramp
TAILORD = 1          # wave-3 diag order variant (tail scheduling A/B)
CH0 = 1              # chunk-0 piece granularity: 0=per-pair, 1=4-tile batches
TRACE = False        # test.py sets True to capture an NTFF profile
LAST_RESULT = None   # BassKernelResults of the most recent run (for test.py)

_nc_cache = {}


def _waves():
    """Wave c: diagonal chunks (m, m*128, 512-128r) of row-quad c plus the
    full 512-wide chunks (m, c*512, 512) of all rows m < 4c."""
    waves = [[] for _ in range(NTILES)]
    for m in range(MT):
        q, r = divmod(m, 4)
        waves[q].append((m, m * P, NT - r * P))
        for n in range(q + 1, NTILES):
            waves[n].append((m, NT * n, NT))
    return waves


def _build_nc():
    key = (DRAIN, WARM, BGP, SMP, TAILORD, CH0)
    if key in _nc_cache:
        return _nc_cache[key]

    import concourse.bass as bass
    import concourse.mybir as mybir
    import concourse.tile as tile
    from concourse import bacc

    f32 = mybir.dt.float32
    bf16 = mybir.dt.bfloat16
    f8 = mybir.dt.float8e4
    DR = mybir.MatmulPerfMode.DoubleRow

    nc = bacc.Bacc("TRN2", target_bir_lowering=False, debug=False, num_devices=8)
    a8b_dram = nc.dram_tensor("a8b", [DBG, L], f8, kind="ExternalInput")
    h8b_dram = nc.dram_tensor("h8b", [DBG, L], f8, kind="ExternalInput")
    ra8_dram = nc.dram_tensor("ra8", [DBG, L], f8, kind="ExternalInput")
    rh8_dram = nc.dram_tensor("rh8", [DBG, L], f8, kind="ExternalInput")
    a8s_dram = nc.dram_tensor("a8s", [DSM, L], f8, kind="ExternalInput")
    h8s_dram = nc.dram_tensor("h8s", [DSM, L], f8, kind="ExternalInput")
    b_dram = nc.dram_tensor("bias", [P, 1], f32, kind="ExternalInput")
    out_dram = nc.dram_tensor("out", [L, L], bf16, kind="ExternalOutput")

    def r3(t):  # [D', L] -> [128, D'/128, L]
        return t[:, :].rearrange("(t p) l -> p t l", p=P)

    a8b3, h8b3, ra83, rh83 = r3(a8b_dram), r3(h8b_dram), r3(ra8_dram), r3(rh8_dram)
    a8s3, h8s3 = r3(a8s_dram), r3(h8s_dram)

    with tile.TileContext(nc) as tc:
        with (
            tc.tile_pool(name="data", bufs=1) as data,
            tc.tile_pool(name="psum", bufs=7, space="PSUM") as psum,
            tc.tile_pool(name="psumw", bufs=1, space="PSUM") as psumw,
            tc.tile_pool(name="stage", bufs=4) as stage,
            tc.tile_pool(name="stagef", bufs=2) as stagef,
        ):
            TBG = 2 * BGP
            a8b_sb = data.tile([P, TBG, L], f8)  # 12KB/partition each
            h8b_sb = data.tile([P, TBG, L], f8)
            ra8_sb = data.tile([P, TBG, L], f8)
            rh8_sb = data.tile([P, TBG, L], f8)
            a8s_sb = data.tile([P, 2 * SMP, L], f8)
            h8s_sb = data.tile([P, 2 * SMP, L], f8)
            b_sb = data.tile([P, 1], f32)
            junk = data.tile([P, NT], bf16)

            # PE p-state warmup: junk matmuls into a scratch psum bank keep
            # the PE busy while chunk 0 loads, so the ~3us ramp to 2.4 GHz
            # happens during the DMA head instead of on real work.
            nc.vector.memset(junk[:, :], 0.0)
            jp = psumw.tile([P, NT], f32, name="jp")
            for _ in range(WARM):
                nc.tensor.matmul(jp, junk[:, :P], junk[:, :], start=True, stop=True)

            b_ap = b_sb[:, 0, None]

            def emit_load_h(jc, pieces=((0, 6),)):
                # h-side: moving operands, needed by wave jc's full groups
                js = bass.ts(jc, NT)
                for t0, t1 in pieces:
                    nc.sync.dma_start(h8b_sb[:, t0:t1, js], h8b3[:, t0:t1, js])
                nc.sync.dma_start(rh8_sb[:, :, js], rh83[:, :, js])
                nc.sync.dma_start(h8s_sb[:, :, js], h8s3[:, :, js])

            def emit_load_a(jc, pieces=((0, 6),)):
                # a-side: stationary operands, needed by wave jc's diagonals
                js = bass.ts(jc, NT)
                for t0, t1 in pieces:
                    nc.sync.dma_start(a8b_sb[:, t0:t1, js], a8b3[:, t0:t1, js])
                nc.sync.dma_start(ra8_sb[:, :, js], ra83[:, :, js])
                nc.sync.dma_start(a8s_sb[:, :, js], a8s3[:, :, js])

            def emit_load0():
                # chunk 0 feeds wave 0 (diagonals only): stream pieces in
                # the order the group k-loop consumes them. Piece size ~2
                # k-tiles keeps transfers at the HWDGE issue cadence.
                js = bass.ts(0, NT)
                if CH0 == 0:
                    for i in range(BGP):
                        ts2 = slice(2 * i, 2 * i + 2)
                        nc.sync.dma_start(h8b_sb[:, ts2, js], h8b3[:, ts2, js])
                        nc.sync.dma_start(a8b_sb[:, ts2, js], a8b3[:, ts2, js])
                        if i == 0:
                            nc.sync.dma_start(b_sb[:, :], b_dram[:, :])
                else:
                    for t0, t1 in ((0, 4), (4, 6)):
                        nc.sync.dma_start(h8b_sb[:, t0:t1, js], h8b3[:, t0:t1, js])
                        nc.sync.dma_start(a8b_sb[:, t0:t1, js], a8b3[:, t0:t1, js])
                    nc.sync.dma_start(b_sb[:, :], b_dram[:, :])
                nc.sync.dma_start(rh8_sb[:, :, js], rh83[:, :, js])
                nc.sync.dma_start(ra8_sb[:, :, js], ra83[:, :, js])
                nc.sync.dma_start(h8s_sb[:, :, js], h8s3[:, :, js])
                nc.sync.dma_start(a8s_sb[:, :, js], a8s3[:, :, js])

            def emit_matmuls(m, s, w):
                ps = psum.tile([P, NT], f32, name="ps")[:, :w]
                mt = bass.ts(m, P)
                cs = bass.ds(s, w)
                for i in range(BGP):  # main pairs first (chunk-0 streaming)
                    ts2 = slice(2 * i, 2 * i + 2)
                    nc.tensor.matmul(
                        ps, a8b_sb[:, ts2, mt], h8b_sb[:, ts2, cs],
                        start=(i == 0), stop=False, perf_mode=DR,
                    )
                for i in range(BGP):  # cross terms (residual corrections)
                    ts2 = slice(2 * i, 2 * i + 2)
                    nc.tensor.matmul(
                        ps, a8b_sb[:, ts2, mt], rh8_sb[:, ts2, cs],
                        start=False, stop=False, perf_mode=DR,
                    )
                    nc.tensor.matmul(
                        ps, ra8_sb[:, ts2, mt], h8b_sb[:, ts2, cs],
                        start=False, stop=False, perf_mode=DR,
                    )
                for i in range(SMP):  # plain-fp8 small-|w| pairs
                    ts2 = slice(2 * i, 2 * i + 2)
                    nc.tensor.matmul(
                        ps, a8s_sb[:, ts2, mt], h8s_sb[:, ts2, cs],
                        start=False, stop=(i == SMP - 1), perf_mode=DR,
                    )
                return ps

            def emit_drain(st, ps, gi):
                # PSUM -> SBUF(bf16) fused with the per-partition bias add
                use_act = DRAIN == "act" or (DRAIN == "split" and gi % 2 == 0)
                if use_act:
                    nc.scalar.activation(
                        st, ps, mybir.ActivationFunctionType.Identity,
                        bias=b_ap,
                    )
                else:
                    nc.vector.tensor_scalar_add(st, ps, b_ap)

            def emit_wave(c, wave, gi):
                full = [g for g in wave if g[0] // 4 < c]
                diag = [g for g in wave if g[0] // 4 == c]
                # last wave: 2-row output batches so drains complete and
                # ship early instead of one 4-row DMA head-of-line blocking
                # the ACT sequencer into the tail
                bsz = 2 if c == NTILES - 1 else 4
                for g0 in range(0, len(full), bsz):
                    sub = full[g0 : g0 + bsz]
                    stf = stagef.tile([P, 4 * NT], bf16, name="stf")[
                        :, : len(sub) * NT
                    ]
                    for i, (m, s, w) in enumerate(sub):
                        ps = emit_matmuls(m, s, w)
                        emit_drain(stf[:, bass.ts(i, NT)], ps, gi)
                        gi += 1
                    m0 = sub[0][0]
                    dst = out_dram[
                        bass.ds(m0 * P, len(sub) * P), bass.ts(c, NT)
                    ].rearrange("(t p) c -> p t c", p=P)
                    nc.scalar.dma_start(
                        dst, stf.rearrange("p (t c) -> p t c", c=NT)
                    )
                if c == NTILES - 1:
                    # tail scheduling: order the final diagonal groups so
                    # each group's drain+DMA chain clears the shared HWDGE
                    # stage during the next group's compute
                    order = {
                        0: [0, 2, 1, 3],  # 12,14,13,15
                        1: [0, 1, 2, 3],  # 12,13,14,15
                        2: [1, 0, 2, 3],  # 13,12,14,15
                    }[TAILORD]
                    diag = [diag[i] for i in order]
                for i, (m, s, w) in enumerate(diag):
                    ps = emit_matmuls(m, s, w)
                    st = stage.tile([P, NT], bf16, name="st")[:, :w]
                    emit_drain(st, ps, gi)
                    gi += 1
                    # the very last output rides the idle SP ring (shorter
                    # DGE delay, and not queued behind ACT's prior DMA)
                    eng = nc.sync if c == NTILES - 1 and i == 3 else nc.scalar
                    eng.dma_start(out_dram[bass.ts(m, P), bass.ds(s, w)], st)
                return gi

            # Emission: each wave's h-side tensors ship one wave ahead (its
            # full-width groups only need moving operands; the stationary
            # a-side is only needed once its diagonal groups run).
            waves = _waves()
            emit_load0()
            emit_load_h(1, pieces=((0, 3), (3, 6)))
            gi = emit_wave(0, waves[0], 0)
            emit_load_a(1)
            emit_load_h(2)
            gi = emit_wave(1, waves[1], gi)
            emit_load_a(2)
            emit_load_h(3)
            gi = emit_wave(2, waves[2], gi)
            emit_load_a(3)
            emit_wave(3, waves[3], gi)

    nc.compile()
    _nc_cache[key] = nc
    return nc


def kernel(hidden_states, W, b):
    global LAST_RESULT
    import ml_dtypes
    import concourse.mybir as mybir
    from concourse.bass_utils import run_bass_kernel_spmd

    bf16 = ml_dtypes.bfloat16
    f8 = mybir.dt.np(mybir.dt.float8e4)
    hidden_states = np.asarray(hidden_states, dtype=np.float32)
    W = np.asarray(W, dtype=np.float32)
    b = np.asarray(b, dtype=np.float32)

    Wp = W[:, :D]                                   # [O, D]
    hT = np.ascontiguousarray(hidden_states.transpose(0, 2, 1))  # [B, D, L] f32

    sc = np.float32(2.0**F8S)
    in_maps = []
    for c in range(8):
        bb, o = divmod(c, 2)
        w = Wp[o]
        perm = np.argsort(-np.abs(w))   # big |w| first -> split-fp8 tiles
        big, small = perm[:DBG], perm[DBG:]
        ab = hT[bb][big] * w[big][:, None] * sc           # [768, L] f32
        hb = hT[bb][big] * (np.float32(1.0) / sc)
        a8b = ab.astype(f8)
        ra8 = (ab - a8b.astype(np.float32)).astype(f8)    # residuals, scale 1
        h8b = hb.astype(f8)
        rh8 = (hb - h8b.astype(np.float32)).astype(f8)
        a8s = (hT[bb][small] * w[small][:, None] * sc).astype(f8)
        h8s = (hT[bb][small] * (np.float32(1.0) / sc)).astype(f8)
        bias = np.full((P, 1), b[o], dtype=np.float32)
        in_maps.append({
            "a8b": a8b, "h8b": h8b, "ra8": ra8, "rh8": rh8,
            "a8s": a8s, "h8s": h8s, "bias": bias,
        })

    nc = _build_nc()
    res = run_bass_kernel_spmd(nc, in_maps, core_ids=list(range(8)), trace=TRACE)
    LAST_RESULT = res

    # Unshard: upcast and mirror the strictly-lower blocks from the
    # computed upper triangle (C is symmetric by construction).
    blockmask = np.arange(MT)[None, :] >= np.arange(MT)[:, None]  # j_blk >= i_blk
    out = np.empty((B, L, L, O), dtype=np.float32)
    for c in range(8):
        bb, o = divmod(c, 2)
        C = np.asarray(res.results[c]["out"]).astype(np.float32)
        M = C.reshape(MT, P, MT, P)
        sym = np.where(blockmask[:, None, :, None], M, M.transpose(2, 3, 0, 1))
        out[bb, :, :, o] = sym.reshape(L, L)
    return out
